# revision 1
# baseline (speedup 1.0000x reference)
"""Trainium2 Bass kernel for the BDH recurrent block (B=8, T=256, d=256, n=1024).

Key reformulation: the scan input v_prev is the *embedding* at each step (the
output v_star is never fed back), so the only recurrences are

  x_t = l1norm(0.97 * x_{t-1} + relu(emb_t @ Dx.T))          (elementwise, n)
  rho_t = 0.97 * rho_{t-1} + ln(emb_t) (x) x_t               (rank-1, d*n)

Both have closed forms:
  x_t  = sum_s C[t,s] * U_s           with U = relu(emb @ Dx.T)  and
         C[t,s] = 0.97^{t-s} / prod_{r=s..t} b_r,  b_r = sum(U_r) + 0.97*[r>0]
         (b_0 = sum(U_0)), computed in log space via a cumulative sum.
  a*_t = rho_{t-1} x_t = sum_{s<t} 0.97^{t-1-s} (x_s . x_t) ln(emb_s)
       = ((X X^T) o D) @ ln(emb)     -- decay-masked attention.

So the whole T-step scan becomes a handful of dense matmuls, one sample per
NeuronCore (data-parallel over B=8 across 8 cores, weights replicated).
"""

import numpy as np

import concourse.bass as bass
import concourse.tile as tile
from concourse import bacc, mybir
from concourse.bass_utils import run_bass_kernel_spmd
from concourse.hw_specs import get_activation_tables

B, T, D, N = 8, 256, 256, 1024
P = 128  # partitions
LN_EPS = 1e-5
DECAY = 0.97
F32 = mybir.dt.float32
F32R = mybir.dt.float32r
AF = mybir.ActivationFunctionType
ALU = mybir.AluOpType

# fp32r runs the PE at 4x the fp32 rate (1 cycle/row at N>=256); inputs are
# fp32 in SBUF, bitcast at the matmul. Used only for the big matmuls.
USE_F32R = True


def _mm(nc, out, lhsT, rhs, start, stop, fast):
    nc.tensor.matmul(out, lhsT, rhs, start=start, stop=stop)


def _build_nc(use_f32r=USE_F32R):
    nc = bacc.Bacc(enable_partition_id=False)
    FDT = F32R if use_f32r else F32

    # packed inputs (few large DMAs, ordered by when the pipeline needs them)
    d_tid = nc.dram_tensor("tid", [P, 2 * P], F32, kind="ExternalInput")  # triu|ident
    d_em2 = nc.dram_tensor("em2", [T, 4 + 2 * D], FDT, kind="ExternalInput")  # sc|emb|embT
    d_DxT = nc.dram_tensor("DxT", [D, N], FDT, kind="ExternalInput")
    d_mask2 = nc.dram_tensor("mask2", [T, 2 * T], F32, kind="ExternalInput")  # maskCT|DupT
    d_DyT = nc.dram_tensor("DyT", [D, N], FDT, kind="ExternalInput")
    d_ET = nc.dram_tensor("ET", [N, D], FDT, kind="ExternalInput")
    d_out = nc.dram_tensor("out", [T, D], F32, kind="ExternalOutput")

    # Preload the one ACT table set containing every function we use
    # (relu/ln/exp/copy) so the compiler never swaps tables mid-kernel
    # (each swap costs ~2.7us on the Scalar engine).
    act_sets = list(get_activation_tables(nc.m.arch))
    combined_set_id = act_sets.index("natural_log_exp_and_others")

    with tile.TileContext(nc) as tc:
        nc.scalar.add_instruction(mybir.InstLoadActFuncSet(
            name=nc.get_next_instruction_name(),
            act_func_set_id=combined_set_id, ins=[], outs=[]))
        with (
            tc.tile_pool(name="consts", bufs=1) as cp,
            tc.tile_pool(name="work", bufs=1) as wp,
            tc.tile_pool(name="ps512", bufs=2, space="PSUM") as ps512,
            tc.tile_pool(name="ps256", bufs=5, space="PSUM") as ps256,
            tc.tile_pool(name="ps_small", bufs=1, space="PSUM") as pss,
        ):
            # ---- load inputs (issue order == need order) --------------------
            def load2(dram, f, tag, dt_=F32):  # (2P, f) dram -> two [P, f] tiles
                ts = []
                for k in range(2):
                    t = cp.tile([P, f], dt_, tag=f"{tag}{k}", name=f"{tag}{k}")
                    nc.sync.dma_start(t[:], dram[k * P:(k + 1) * P, :])
                    ts.append(t)
                return ts

            def load_split(dram, f, tag, dt_=F32):
                # tile k=0 issued on SP, k=1 on ACT (both are HWDGE-capable)
                ts = []
                for k, eng in ((0, nc.sync), (1, nc.scalar)):
                    t = cp.tile([P, f], dt_, tag=f"{tag}{k}", name=f"{tag}{k}")
                    eng.dma_start(t[:], dram[k * P:(k + 1) * P, :])
                    ts.append(t)
                return ts

            em2_s = load_split(d_em2, 4 + 2 * D, "em2", FDT)
            c097_s = [t[:, 0:1].bitcast(F32) for t in em2_s]
            iotaP_s = [t[:, 1:2].bitcast(F32) for t in em2_s]
            iotaQ_s = [t[:, 2:3].bitcast(F32) for t in em2_s]
            emb_s = [t[:, 4:4 + D].bitcast(F32) for t in em2_s]
            embT_s = [t[:, 4 + D:4 + 2 * D] for t in em2_s]
            DxT_c = {}
            for k, eng in ((0, nc.sync), (1, nc.scalar)):
                for ch in range(2):
                    t = cp.tile([P, 512], FDT, tag=f"DxT{k}{ch}",
                                name=f"DxT{k}{ch}")
                    eng.dma_start(
                        t[:], d_DxT[k * P:(k + 1) * P, ch * 512:(ch + 1) * 512])
                    DxT_c[(k, ch)] = t
            mask2_s = load_split(d_mask2, 2 * T, "mask2")
            maskCT_s = [t[:, 0:T] for t in mask2_s]
            DupT_s = [t[:, T:2 * T] for t in mask2_s]
            tid_s = cp.tile([P, 2 * P], F32, tag="tid", name="tid")
            nc.sync.dma_start(tid_s[:], d_tid[:, :])
            triu_s = tid_s[:, 0:P]
            ident_s = tid_s[:, P:2 * P]
            DyT_s = load_split(d_DyT, N, "DyT", FDT)
            et_big = cp.tile([P, 8, D], FDT, tag="et_big", name="et_big")
            nc.sync.dma_start(
                et_big[:], d_ET.rearrange("(k p) d -> p k d", p=P))
            ET_s = [et_big[:, k, :] for k in range(8)]
            ones_blk = cp.tile([P, P], F32, tag="ones_blk", name="ones_blk")
            nc.vector.memset(ones_blk[:], 1.0)


            ones_row = cp.tile([1, P], F32, tag="ones_row", name="ones_row")
            nc.vector.memset(ones_row[:], 1.0)
            zero_col = cp.tile([P, 1], F32, tag="zero_col", name="zero_col")
            nc.vector.memset(zero_col[:], 0.0)
            eps_col = cp.tile([P, 1], F32, tag="eps_col", name="eps_col")
            nc.vector.memset(eps_col[:], LN_EPS)

            # ---- U = relu(emb @ Dx.T), row sums a ---------------------------
            U_s = [wp.tile([P, N], FDT, tag=f"U{m}", name=f"U{m}") for m in range(2)]
            a_s = [wp.tile([P, 1], F32, tag=f"a{m}", name=f"a{m}") for m in range(2)]
            apart = [[wp.tile([P, 1], F32, tag=f"ap{m}{c}", name=f"ap{m}{c}") for c in range(2)]
                     for m in range(2)]
            for mt in range(2):
                for ch in range(2):
                    pu = ps512.tile([P, 512], F32, tag="pu", name="pu")
                    for k in range(2):
                        _mm(nc, pu[:], embT_s[k][:, mt * P:(mt + 1) * P],
                            DxT_c[(k, ch)][:],
                            start=(k == 0), stop=(k == 1), fast=use_f32r)
                    if ch == 0:
                        nc.scalar.activation(
                            out=U_s[mt][:, ch * 512:(ch + 1) * 512], in_=pu[:],
                            func=AF.Relu, bias=zero_col[:],
                            accum_out=apart[mt][ch][:])
                    else:
                        nc.vector.tensor_scalar(
                            U_s[mt][:, ch * 512:(ch + 1) * 512], pu[:], 0.0,
                            0.0, op0=ALU.max, op1=ALU.add,
                            accum_out=apart[mt][ch][:])

            # ---- scalar chain: b, log b, cumsum, p, q ------------------------
            logb_s = []
            q_s = []
            p_s = []
            for mt in range(2):
                bvec = wp.tile([P, 1], F32, tag=f"b{mt}", name=f"b{mt}")
                nc.vector.scalar_tensor_tensor(
                    out=bvec[:], in0=apart[mt][0][:], scalar=c097_s[mt][:],
                    in1=apart[mt][1][:], op0=ALU.add, op1=ALU.add)
                lb = wp.tile([P, 1], F32, tag=f"lb{mt}", name=f"lb{mt}")
                nc.scalar.activation(out=lb[:], in_=bvec[:], func=AF.Ln, bias=zero_col[:])
                logb_s.append(lb)
            for mt in range(2):
                pl = pss.tile([P, 1], F32, tag="pss", name="plam")
                if mt == 0:
                    nc.tensor.matmul(pl[:], triu_s[:], logb_s[0][:],
                                     start=True, stop=True)
                else:
                    nc.tensor.matmul(pl[:], ones_blk[:], logb_s[0][:],
                                     start=True, stop=False)
                    nc.tensor.matmul(pl[:], triu_s[:], logb_s[1][:],
                                     start=False, stop=True)
                # q = lamS + iotaQ ; p = (iotaP - lamS) - logb   (lamS in PSUM)
                qv = wp.tile([P, 1], F32, tag=f"q{mt}", name=f"q{mt}")
                nc.vector.tensor_add(qv[:], pl[:], iotaQ_s[mt][:])
                q_s.append(qv)
                pv = wp.tile([P, 1], F32, tag=f"p{mt}", name=f"p{mt}")
                nc.vector.scalar_tensor_tensor(
                    out=pv[:], in0=iotaP_s[mt][:], scalar=pl[:],
                    in1=logb_s[mt][:], op0=ALU.subtract, op1=ALU.subtract)
                p_s.append(pv)

            # ---- p as a row vector (PE transpose) ---------------------------
            p_row = wp.tile([1, T], F32, tag="p_row", name="p_row")
            for mt in range(2):
                pt = pss.tile([1, P], F32, tag="pss", name="ptr")
                nc.tensor.transpose(pt[:], p_s[mt][:], ident_s[:])
                nc.vector.tensor_copy(p_row[:, mt * P:(mt + 1) * P], pt[:])

            # ---- CT[s,t] = exp(q_s + p_t + mask) ----------------------------
            CT_s = []
            for st in range(2):
                pb = ps256.tile([P, T], F32, tag="ps", name="pb")
                nc.tensor.matmul(pb[:], ones_row[:], p_row[:],
                                 start=True, stop=True)
                tmp = wp.tile([P, T], F32, tag=f"ctmp{st}", name=f"ctmp{st}")
                nc.vector.tensor_add(tmp[:], pb[:], maskCT_s[st][:])
                ct = wp.tile([P, T], FDT, tag=f"CT{st}", name=f"CT{st}")
                nc.scalar.activation(out=ct[:], in_=tmp[:], func=AF.Exp,
                                     bias=q_s[st][:], scale=1.0)
                CT_s.append(ct)

            # ---- X^T = U^T C^T  (n on partitions, T free) -------------------
            XT_s = []
            for m in range(8):
                px = ps256.tile([P, T], F32, tag="ps", name="px")
                for k in range(2):
                    _mm(nc, px[:], U_s[k][:, m * P:(m + 1) * P], CT_s[k][:],
                        start=(k == 0), stop=(k == 1), fast=use_f32r)
                xt = wp.tile([P, T], FDT, tag=f"XT{m}", name=f"XT{m}")
                if m % 2 == 0:
                    nc.vector.tensor_copy(xt[:], px[:])
                else:
                    nc.scalar.copy(xt[:], px[:])
                XT_s.append(xt)

            # ---- W = ln(emb rows) -------------------------------------------
            W_s = []
            for mt in range(2):
                st6 = wp.tile([P, 6], F32, tag=f"wst{mt}", name=f"wst{mt}")
                nc.vector.bn_stats(st6[:], emb_s[mt][:])
                mv = wp.tile([P, 2], F32, tag=f"wmv{mt}", name=f"wmv{mt}")
                nc.vector.bn_aggr(mv[:], st6[:])
                lv = wp.tile([P, 1], F32, tag=f"wlv{mt}", name=f"wlv{mt}")
                nc.scalar.activation(out=lv[:], in_=mv[:, 1:2], func=AF.Ln,
                                     bias=eps_col[:])
                rs = wp.tile([P, 1], F32, tag=f"wrs{mt}", name=f"wrs{mt}")
                nc.scalar.activation(out=rs[:], in_=lv[:], func=AF.Exp,
                                     bias=zero_col[:], scale=-0.5)
                w = wp.tile([P, D], FDT, tag=f"W{mt}", name=f"W{mt}")
                nc.vector.tensor_scalar(w[:], emb_s[mt][:], mv[:, 0:1], rs[:],
                                        op0=ALU.subtract, op1=ALU.mult)
                W_s.append(w)

            # ---- G = X X^T ; GD = G o Dup -----------------------------------
            GD_s = []
            for st in range(2):
                pg = ps256.tile([P, T], F32, tag="ps", name="pg")
                for k in range(8):
                    _mm(nc, pg[:], XT_s[k][:, st * P:(st + 1) * P], XT_s[k][:],
                        start=(k == 0), stop=(k == 7), fast=use_f32r)
                gd = wp.tile([P, T], FDT, tag=f"GD{st}", name=f"GD{st}")
                nc.vector.tensor_mul(gd[:], pg[:], DupT_s[st][:])
                GD_s.append(gd)

            # ---- A = (G o D) @ W  ([t, d]) + layernorm ----------------------
            Aln_s = []
            AlnT_s = [wp.tile([P, T], FDT, tag=f"AlnT{k}", name=f"AlnT{k}")
                      for k in range(2)]
            for mt in range(2):
                pa = ps256.tile([P, D], F32, tag="ps", name="pa")
                ks = [0] if mt == 0 else [0, 1]
                for k in ks:
                    _mm(nc, pa[:], GD_s[k][:, mt * P:(mt + 1) * P], W_s[k][:],
                        start=(k == ks[0]), stop=(k == ks[-1]), fast=use_f32r)
                st6 = wp.tile([P, 6], F32, tag=f"ast{mt}", name=f"ast{mt}")
                nc.vector.bn_stats(st6[:], pa[:])
                mv = wp.tile([P, 2], F32, tag=f"amv{mt}", name=f"amv{mt}")
                nc.vector.bn_aggr(mv[:], st6[:])
                lv = wp.tile([P, 1], F32, tag=f"alv{mt}", name=f"alv{mt}")
                nc.scalar.activation(out=lv[:], in_=mv[:, 1:2], func=AF.Ln,
                                     bias=eps_col[:])
                rs = wp.tile([P, 1], F32, tag=f"ars{mt}", name=f"ars{mt}")
                nc.scalar.activation(out=rs[:], in_=lv[:], func=AF.Exp,
                                     bias=zero_col[:], scale=-0.5)
                al = wp.tile([P, D], F32, tag=f"Aln{mt}", name=f"Aln{mt}")
                nc.vector.tensor_scalar(al[:], pa[:], mv[:, 0:1], rs[:],
                                        op0=ALU.subtract, op1=ALU.mult)
                Aln_s.append(al)

            # ---- Aln^T via PE transpose ([d, t]) ----------------------------
            for mt in range(2):
                for dt_ in range(2):
                    ptr = ps256.tile([P, P], F32, tag="ps", name="atr")
                    nc.tensor.transpose(ptr[:], Aln_s[mt][:, dt_ * P:(dt_ + 1) * P],
                                        ident_s[:])
                    nc.vector.tensor_copy(
                        AlnT_s[dt_][:, mt * P:(mt + 1) * P], ptr[:])

            # ---- y^T = relu(Dy ln(A)^T) o X^T -------------------------------
            yT_s = []
            for m in range(8):
                py = ps256.tile([P, T], F32, tag="ps", name="py")
                for k in range(2):
                    _mm(nc, py[:], DyT_s[k][:, m * P:(m + 1) * P], AlnT_s[k][:],
                        start=(k == 0), stop=(k == 1), fast=use_f32r)
                yt = wp.tile([P, T], FDT, tag=f"yT{m}", name=f"yT{m}")
                nc.vector.scalar_tensor_tensor(
                    out=yt[:], in0=py[:], scalar=0.0, in1=XT_s[m][:].bitcast(F32),
                    op0=ALU.max, op1=ALU.mult)
                yT_s.append(yt)

            # ---- v = y E^T ([t, d]) + layernorm + store ---------------------
            for mt in range(2):
                pv = ps256.tile([P, D], F32, tag="ps", name="pv")
                for k in range(8):
                    _mm(nc, pv[:], yT_s[k][:, mt * P:(mt + 1) * P], ET_s[k][:],
                        start=(k == 0), stop=(k == 7), fast=use_f32r)
                st6 = wp.tile([P, 6], F32, tag=f"ost{mt}", name=f"ost{mt}")
                nc.vector.bn_stats(st6[:], pv[:])
                mv = wp.tile([P, 2], F32, tag=f"omv{mt}", name=f"omv{mt}")
                nc.vector.bn_aggr(mv[:], st6[:])
                lv = wp.tile([P, 1], F32, tag=f"olv{mt}", name=f"olv{mt}")
                nc.scalar.activation(out=lv[:], in_=mv[:, 1:2], func=AF.Ln,
                                     bias=eps_col[:])
                rs = wp.tile([P, 1], F32, tag=f"ors{mt}", name=f"ors{mt}")
                nc.scalar.activation(out=rs[:], in_=lv[:], func=AF.Exp,
                                     bias=zero_col[:], scale=-0.5)
                ov = wp.tile([P, D], F32, tag=f"ov{mt}", name=f"ov{mt}")
                nc.vector.tensor_scalar(ov[:], pv[:], mv[:, 0:1], rs[:],
                                        op0=ALU.subtract, op1=ALU.mult)
                nc.sync.dma_start(d_out[mt * P:(mt + 1) * P, :], ov[:])

    nc.finalize()
    return nc


_NC_CACHE = {}


def _get_nc(use_f32r=USE_F32R):
    if use_f32r not in _NC_CACHE:
        _NC_CACHE[use_f32r] = _build_nc(use_f32r)
    return _NC_CACHE[use_f32r]


def _host_consts():
    ii = np.arange(T, dtype=np.float64)
    ln097 = np.log(np.float64(DECAY))
    maskCT = np.where(ii[:, None] <= ii[None, :], 0.0, -1e30).astype(np.float32)
    DupT = np.where(
        ii[:, None] < ii[None, :],
        np.float64(DECAY) ** (ii[None, :] - 1 - ii[:, None]),
        0.0,
    ).astype(np.float32)
    mask2 = np.ascontiguousarray(np.concatenate([maskCT, DupT], axis=1))
    tid = np.ascontiguousarray(np.concatenate(
        [np.triu(np.ones((P, P), np.float32), k=1), np.eye(P, dtype=np.float32)],
        axis=1))
    sc = np.zeros((T, 4), np.float32)
    sc[:, 0] = DECAY
    sc[0, 0] = 0.0
    sc[:, 1] = (ii * ln097).astype(np.float32)
    sc[:, 2] = (-ii * ln097).astype(np.float32)
    return sc, tid, mask2


def make_in_maps(embeddings, E, Dx, Dy):
    emb = np.ascontiguousarray(np.asarray(embeddings, dtype=np.float32))
    E = np.asarray(E, dtype=np.float32)
    Dx = np.asarray(Dx, dtype=np.float32)
    Dy = np.asarray(Dy, dtype=np.float32)
    sc, tid, mask2 = _host_consts()
    shared = {
        "tid": tid, "mask2": mask2,
        "DxT": np.ascontiguousarray(Dx.T),
        "DyT": np.ascontiguousarray(Dy.T),
        "ET": np.ascontiguousarray(E.T),
    }
    in_maps = []
    for b in range(B):
        m = dict(shared)
        m["em2"] = np.ascontiguousarray(
            np.concatenate([sc, emb[b], emb[b].T], axis=1))
        in_maps.append(m)
    return in_maps


def kernel(embeddings, E, Dx, Dy, _use_f32r=USE_F32R):
    in_maps = make_in_maps(embeddings, E, Dx, Dy)
    nc = _get_nc(_use_f32r)
    res = run_bass_kernel_spmd(nc, in_maps, core_ids=list(range(B)))
    return np.stack([r["out"] for r in res.results], axis=0)



# revision 10
# speedup vs baseline: 1.0307x; 1.0307x over previous
"""Trainium2 Bass kernel for the BDH recurrent block (B=8, T=256, d=256, n=1024).

One sample per NeuronCore (data-parallel over B=8), weights replicated.

The scan input v_prev is the *embedding* at each step (v_star is never fed
back), so the only recurrences are

  x_t  = (0.97 x_{t-1} + relu(emb_t Dx^T)) / b_t,  b_t = sum(U_t) + 0.97[t>0]
  rho_t = 0.97 rho_{t-1} + ln(emb_t) (x) x_t

Key implementation ideas (vs the straightforward masked-attention form):

 * The x recurrence is a first-order linear recurrence along t:
   z_t = (0.97/b_{t-1}) z_{t-1} + U_t, x_t = z_t / b_t.  The DVE/GpSimd
   `tensor_tensor_scan` instruction computes it exactly in X^T layout
   [n_partitions, t_free] -- no log-space cumsum / exp / masks needed.
 * a*_t = rho_{t-1} x_t = ((X X^T) o Dup) @ ln(emb): decay-masked attention.
 * ln(a*) mean is exactly 0 (rows of ln(emb) are zero-mean), so the A-layernorm
   reduces to a per-row scale r_t = rsqrt(var(a*)+eps), and since
   relu(r*c) = r*relu(c) for r>0, r_t commutes out to the final v matmul
   output, where it is applied before the (eps-sensitive) output layernorm.
 * A^T (for the Dy matmul) is computed by a second small matmul instead of
   PE transposes.
 * bf16 storage for all big operands (halves HBM traffic, which bounds the
   front of the kernel: 8 cores share HBM).
"""

import numpy as np
import ml_dtypes

import concourse.bass as bass
import concourse.tile as tile
from concourse import bacc, mybir
from concourse.bass_utils import run_bass_kernel_spmd
from concourse.hw_specs import get_activation_tables

B, T, D, N = 8, 256, 256, 1024
P = 128
LN_EPS = 1e-5
DECAY = 0.97
F32 = mybir.dt.float32
F32R = mybir.dt.float32r
BF16 = mybir.dt.bfloat16
AF = mybir.ActivationFunctionType
ALU = mybir.AluOpType
NPBF16 = ml_dtypes.bfloat16

# filler-matmul counts keeping the PE busy (HAM clock-gate warm) during
# DMA waits / vector-chain gaps; tuned from the perfetto trace
FILL_PRE = 14
FILL_B = 2
FILL_SCAN = 5
FILL_GD = 3
FILL_AT = 2


def _build_nc():
    nc = bacc.Bacc(enable_partition_id=False)

    d_embdx0 = nc.dram_tensor("embdx0", [P, 1280], BF16, kind="ExternalInput")
    d_embdx1 = nc.dram_tensor("embdx1", [P, 1280], BF16, kind="ExternalInput")
    d_et = nc.dram_tensor("et", [P, 2048], BF16, kind="ExternalInput")
    d_dyt = nc.dram_tensor("dyt", [P, 2048], BF16, kind="ExternalInput")
    d_emb = nc.dram_tensor("emb", [P, 512], BF16, kind="ExternalInput")
    d_dup = nc.dram_tensor("dup", [P, 384], F32, kind="ExternalInput")
    d_out = nc.dram_tensor("out", [T, D], F32, kind="ExternalOutput")

    act_sets = list(get_activation_tables(nc.m.arch))
    combined_set_id = act_sets.index("natural_log_exp_and_others")

    with tile.TileContext(nc) as tc:
        with (
            tc.tile_pool(name="consts", bufs=1) as cp,
            tc.tile_pool(name="work", bufs=1) as wp,
            tc.tile_pool(name="psA", bufs=7, space="PSUM") as psA,
            tc.tile_pool(name="psF", bufs=1, space="PSUM") as psF,
        ):
            # ---- input DMAs (3 queues; first chunks gate the U matmuls) ----
            embdx0 = cp.tile([P, 1280], BF16, tag="embdx0", name="embdx0")
            embdx1 = cp.tile([P, 1280], BF16, tag="embdx1", name="embdx1")
            et_big = cp.tile([P, 2048], BF16, tag="et", name="et")
            dyt = cp.tile([P, 2048], BF16, tag="dyt", name="dyt")
            embp = cp.tile([P, 512], BF16, tag="embp", name="embp")
            dup = cp.tile([P, 384], F32, tag="dup", name="dup")
            nc.sync.dma_start(embdx0[:], d_embdx0[:, :])
            nc.scalar.dma_start(embdx1[:], d_embdx1[:, :])
            nc.sync.dma_start(et_big[:], d_et[:, :])
            nc.scalar.dma_start(dyt[:], d_dyt[:, :])
            nc.gpsimd.dma_start(embp[:], d_emb[:, :])
            nc.gpsimd.dma_start(dup[:], d_dup[:, :])

            nc.scalar.add_instruction(mybir.InstLoadActFuncSet(
                name=nc.get_next_instruction_name(),
                act_func_set_id=combined_set_id, ins=[], outs=[]))

            embT = [embdx0[:, 0:T], embdx1[:, 0:T]]
            DxT = [embdx0[:, T:1280], embdx1[:, T:1280]]
            DyT = [dyt[:, 0:N], dyt[:, N:2 * N]]
            ET = [et_big[:, m * D:(m + 1) * D] for m in range(8)]
            emb_s = [embp[:, 0:D], embp[:, D:2 * D]]
            Dup0 = dup[:, 0:T]
            Dup1r = dup[:, T:T + P]

            # ---- small consts via memset --------------------------------
            ones_col = cp.tile([P, 1], BF16, tag="ones_col", name="ones_col")
            nc.vector.memset(ones_col[:], 1.0)
            ones_row = cp.tile([1, P], BF16, tag="ones_row", name="ones_row")
            nc.vector.memset(ones_row[:], 1.0)
            c097_1 = cp.tile([1, 1], BF16, tag="c097_1", name="c097_1")
            nc.vector.memset(c097_1[:], DECAY)
            mask_row = cp.tile([1, T], BF16, tag="mask_row", name="mask_row")
            nc.vector.memset(mask_row[:], 1.0)
            nc.vector.memset(mask_row[:, 0:1], 0.0)
            eps_col = cp.tile([P, 1], F32, tag="eps_col", name="eps_col")
            nc.vector.memset(eps_col[:], LN_EPS)
            zero_col = cp.tile([P, 1], F32, tag="zero_col", name="zero_col")
            nc.vector.memset(zero_col[:], 0.0)
            scr_l = cp.tile([P, P], BF16, tag="scr_l", name="scr_l")
            nc.gpsimd.memset(scr_l[:], 0.25)
            scr_r = cp.tile([P, T], BF16, tag="scr_r", name="scr_r")
            nc.gpsimd.memset(scr_r[:], 0.25)
            GD1 = wp.tile([P, T], BF16, tag="GD1", name="GD1")
            nc.gpsimd.memset(GD1[:], 0.0)

            fill_ps = psF.tile([P, T], F32, tag="fill", name="fill")

            def fillers(k):
                for _ in range(k):
                    nc.tensor.matmul(fill_ps[:], scr_l[:], scr_r[:],
                                     start=True, stop=True)

            fillers(FILL_PRE)

            # ---- U^T = relu(Dx emb^T) in [n_part, t] layout -------------
            UT = [wp.tile([P, T], BF16, tag=f"UT{m}", name=f"UT{m}")
                  for m in range(8)]
            pu_tiles = []
            for m in range(8):
                pu = psA.tile([P, T], F32, tag="ps", name=f"pu{m}")
                for k in range(2):
                    nc.tensor.matmul(pu[:], DxT[k][:, m * P:(m + 1) * P],
                                     embT[k][:], start=(k == 0), stop=(k == 1))
                pu_tiles.append(pu)
            for m in range(8):
                pu = pu_tiles[m]
                if m % 2 == 0:
                    nc.scalar.activation(out=UT[m][:], in_=pu[:],
                                         func=AF.Relu, bias=zero_col[:])
                else:
                    nc.vector.tensor_scalar(UT[m][:], pu[:], 0.0, 0.0,
                                            op0=ALU.max, op1=ALU.add)

            # ---- b_t = sum_n U_t + 0.97[t>0]  (row via ones matmul) -----
            b_ps = psA.tile([1, T], F32, tag="ps", name="b")
            for m in range(8):
                nc.tensor.matmul(b_ps[:], ones_col[:], UT[m][:],
                                 start=(m == 0), stop=False)
            nc.tensor.matmul(b_ps[:], c097_1[:], mask_row[:],
                             start=False, stop=True)
            fillers(FILL_B)

            # ---- W = ln(emb rows)  (overlaps U phase) -------------------
            W = []
            for mt in range(2):
                st6 = wp.tile([P, 6], F32, tag=f"wst{mt}", name=f"wst{mt}")
                nc.vector.bn_stats(st6[:], emb_s[mt])
                mv = wp.tile([P, 2], F32, tag=f"wmv{mt}", name=f"wmv{mt}")
                nc.vector.bn_aggr(mv[:], st6[:])
                lv = wp.tile([P, 1], F32, tag=f"wlv{mt}", name=f"wlv{mt}")
                nc.scalar.activation(out=lv[:], in_=mv[:, 1:2], func=AF.Ln,
                                     bias=eps_col[:])
                rs = wp.tile([P, 1], F32, tag=f"wrs{mt}", name=f"wrs{mt}")
                nc.scalar.activation(out=rs[:], in_=lv[:], func=AF.Exp,
                                     bias=zero_col[:], scale=-0.5)
                w = wp.tile([P, D], BF16, tag=f"W{mt}", name=f"W{mt}")
                nc.gpsimd.tensor_scalar(w[:], emb_s[mt], mv[:, 0:1], rs[:],
                                        op0=ALU.subtract, op1=ALU.mult)
                W.append(w)

            # ---- scan coefficients: recip = 1/b, coef_t = 0.97/b_{t-1} --
            lb_row = wp.tile([1, T], F32, tag="lb_row", name="lb_row")
            nc.scalar.activation(out=lb_row[:], in_=b_ps[:], func=AF.Ln,
                                 bias=zero_col[0:1, :])
            recip_bf = wp.tile([1, T], BF16, tag="recip", name="recip")
            nc.scalar.activation(out=recip_bf[:], in_=lb_row[:], func=AF.Exp,
                                 bias=zero_col[0:1, :], scale=-1.0)
            coefp_bf = wp.tile([1, T], BF16, tag="coefp", name="coefp")
            nc.vector.memset(coefp_bf[:, 0:1], 0.0)
            nc.gpsimd.tensor_scalar(coefp_bf[:, 1:T], recip_bf[:, 0:T - 1],
                                    DECAY, 0.0, op0=ALU.mult, op1=ALU.add)
            # broadcast rows to all 128 partitions (ones matmul)
            coefB_ps = psA.tile([P, T], F32, tag="ps", name="coefB")
            nc.tensor.matmul(coefB_ps[:], ones_row[:], coefp_bf[:],
                             start=True, stop=True)
            recipB_ps = psA.tile([P, T], F32, tag="ps", name="recipB")
            nc.tensor.matmul(recipB_ps[:], ones_row[:], recip_bf[:],
                             start=True, stop=True)
            fillers(FILL_SCAN)
            coefB = wp.tile([P, T], BF16, tag="coefB_sb", name="coefB_sb")
            nc.vector.tensor_copy(coefB[:], coefB_ps[:])
            recipB_sb = wp.tile([P, T], BF16, tag="recipB_sb",
                                name="recipB_sb")
            nc.vector.tensor_copy(recipB_sb[:], recipB_ps[:])

            # ---- z scan + x = z/b:  X^T tiles [n_part, t] ---------------
            z = [wp.tile([P, T], BF16, tag=f"z{m}", name=f"z{m}")
                 for m in range(8)]
            XT = [wp.tile([P, T], F32R, tag=f"XT{m}", name=f"XT{m}")
                  for m in range(8)]
            for m in range(8):
                nc.vector.tensor_tensor_scan(out=z[m][:], data0=coefB[:],
                                             data1=UT[m][:], initial=0.0,
                                             op0=ALU.mult, op1=ALU.add)
                nc.gpsimd.tensor_tensor(XT[m][:], z[m][:], recipB_sb[:],
                                        op=ALU.mult)

            # ---- G = X X^T ; GD = G o Dup -------------------------------
            pg0 = psA.tile([P, T], F32, tag="ps", name="pg0")
            pg1 = psA.tile([P, T], F32, tag="ps", name="pg1")
            for m in range(8):
                nc.tensor.matmul(pg0[:], XT[m][:, 0:P], XT[m][:],
                                 start=(m == 0), stop=(m == 7))
                nc.tensor.matmul(pg1[:], XT[m][:, P:T], XT[m][:],
                                 start=(m == 0), stop=(m == 7))
            fillers(FILL_GD)
            GD0 = wp.tile([P, T], BF16, tag="GD0", name="GD0")
            nc.vector.tensor_tensor(GD0[:], pg0[:], Dup0, op=ALU.mult)
            nc.vector.tensor_tensor(GD1[:, P:T], pg1[:, P:T], Dup1r,
                                    op=ALU.mult)

            # ---- A^T directly (no transposes); pa for var only ----------
            ATp = [psA.tile([P, T], F32, tag="ps", name=f"ATp{dt}")
                   for dt in range(2)]
            for dt in range(2):
                nc.tensor.matmul(ATp[dt][:], W[0][:, dt * P:(dt + 1) * P],
                                 GD0[:], start=True, stop=False)
                nc.tensor.matmul(ATp[dt][:], W[1][:, dt * P:(dt + 1) * P],
                                 GD1[:], start=False, stop=True)
            pa = [psA.tile([P, D], F32, tag="ps", name=f"pa{mt}")
                  for mt in range(2)]
            nc.tensor.matmul(pa[0][:], GD0[:, 0:P], W[0][:],
                             start=True, stop=True)
            nc.tensor.matmul(pa[1][:], GD0[:, P:T], W[0][:],
                             start=True, stop=False)
            nc.tensor.matmul(pa[1][:], GD1[:, P:T], W[1][:],
                             start=False, stop=True)
            fillers(FILL_AT)

            AT = [wp.tile([P, T], BF16, tag=f"AT{dt}", name=f"AT{dt}")
                  for dt in range(2)]
            nc.scalar.copy(AT[0][:], ATp[0][:])
            nc.vector.tensor_copy(AT[1][:], ATp[1][:])

            # r_t = rsqrt(var(a*_t) + eps); mean(a*) == 0 analytically, so
            # var = sum(a^2)/D via ACT Square+accumulate (no DVE bn_stats)
            r_col = []
            for mt in range(2):
                sq = wp.tile([P, D], F32, tag=f"asq{mt}", name=f"asq{mt}")
                ss = wp.tile([P, 1], F32, tag=f"ass{mt}", name=f"ass{mt}")
                nc.scalar.activation(out=sq[:], in_=pa[mt][:], func=AF.Square,
                                     bias=zero_col[:], accum_out=ss[:])
                lv = wp.tile([P, 1], F32, tag=f"alv{mt}", name=f"alv{mt}")
                nc.scalar.activation(out=lv[:], in_=ss[:], func=AF.Ln,
                                     bias=eps_col[:], scale=1.0 / D)
                rr = wp.tile([P, 1], F32, tag=f"ar{mt}", name=f"ar{mt}")
                nc.scalar.activation(out=rr[:], in_=lv[:], func=AF.Exp,
                                     bias=zero_col[:], scale=-0.5)
                r_col.append(rr)
            r2_col = []
            for mt in range(2):
                r2 = wp.tile([P, 1], F32, tag=f"r2{mt}", name=f"r2{mt}")
                nc.vector.tensor_tensor(r2[:], r_col[mt][:], r_col[mt][:],
                                        op=ALU.mult)
                r2_col.append(r2)

            # ---- y^T = relu(Dy A^T) o X^T -------------------------------
            yT = [wp.tile([P, T], BF16, tag=f"yT{m}", name=f"yT{m}")
                  for m in range(8)]
            py_tiles = []
            for m in range(8):
                py = psA.tile([P, T], F32, tag="ps", name=f"py{m}")
                for k in range(2):
                    nc.tensor.matmul(py[:], DyT[k][:, m * P:(m + 1) * P],
                                     AT[k][:], start=(k == 0), stop=(k == 1))
                py_tiles.append(py)
                if m % 2 == 0:
                    yb = wp.tile([P, T], F32, tag=f"yb{m}", name=f"yb{m}")
                    nc.scalar.activation(out=yb[:], in_=py[:], func=AF.Relu,
                                         bias=zero_col[:])
                    nc.gpsimd.tensor_tensor(yT[m][:], yb[:],
                                            XT[m][:].bitcast(F32),
                                            op=ALU.mult)
                else:
                    nc.vector.scalar_tensor_tensor(
                        out=yT[m][:], in0=py[:], scalar=0.0,
                        in1=XT[m][:].bitcast(F32),
                        op0=ALU.max, op1=ALU.mult)

            # ---- v = y E^T; apply r_t; output layernorm -----------------
            for mt in range(2):
                pv = psA.tile([P, D], F32, tag="ps", name=f"pv{mt}")
                for m in range(8):
                    nc.tensor.matmul(pv[:], yT[m][:, mt * P:(mt + 1) * P],
                                     ET[m], start=(m == 0), stop=(m == 7))
                st6 = wp.tile([P, 6], F32, tag=f"ost{mt}", name=f"ost{mt}")
                nc.vector.bn_stats(st6[:], pv[:])
                mv = wp.tile([P, 2], F32, tag=f"omv{mt}", name=f"omv{mt}")
                nc.vector.bn_aggr(mv[:], st6[:])
                # out = (pv - mean) * r * rsqrt(r^2 var + eps)
                t1 = wp.tile([P, 1], F32, tag=f"ot1{mt}", name=f"ot1{mt}")
                nc.vector.tensor_tensor(t1[:], mv[:, 1:2], r2_col[mt][:],
                                        op=ALU.mult)
                lv = wp.tile([P, 1], F32, tag=f"olv{mt}", name=f"olv{mt}")
                nc.scalar.activation(out=lv[:], in_=t1[:], func=AF.Ln,
                                     bias=eps_col[:])
                rq = wp.tile([P, 1], F32, tag=f"orq{mt}", name=f"orq{mt}")
                nc.scalar.activation(out=rq[:], in_=lv[:], func=AF.Exp,
                                     bias=zero_col[:], scale=-0.5)
                s = wp.tile([P, 1], F32, tag=f"os{mt}", name=f"os{mt}")
                nc.vector.tensor_tensor(s[:], rq[:], r_col[mt][:],
                                        op=ALU.mult)
                ov = wp.tile([P, D], F32, tag=f"ov{mt}", name=f"ov{mt}")
                nc.vector.tensor_scalar(ov[:], pv[:], mv[:, 0:1], s[:],
                                        op0=ALU.subtract, op1=ALU.mult)
                eng_out = nc.sync if mt == 0 else nc.scalar
                eng_out.dma_start(d_out[mt * P:(mt + 1) * P, :], ov[:])

    nc.finalize()
    return nc


_NC_CACHE = {}


def _get_nc(use_f32r=True):
    if "nc" not in _NC_CACHE:
        _NC_CACHE["nc"] = _build_nc()
    return _NC_CACHE["nc"]


def _host_consts():
    ii = np.arange(T, dtype=np.float64)
    DupT = np.where(
        ii[:, None] < ii[None, :],
        np.float64(DECAY) ** (ii[None, :] - 1 - ii[:, None]),
        0.0,
    ).astype(np.float32)
    dup_pack = np.ascontiguousarray(
        np.concatenate([DupT[0:P, 0:T], DupT[P:T, P:T]], axis=1))
    return dup_pack


def make_in_maps(embeddings, E, Dx, Dy):
    emb = np.asarray(embeddings, dtype=np.float32)
    E = np.asarray(E, dtype=np.float32)
    Dx = np.asarray(Dx, dtype=np.float32)
    Dy = np.asarray(Dy, dtype=np.float32)
    dup_pack = _host_consts()
    DxT = Dx.T  # [d, n]
    DyTp = np.ascontiguousarray(
        Dy.T.reshape(2, P, N).transpose(1, 0, 2).reshape(P, 2 * N)
    ).astype(NPBF16)
    ETp = np.ascontiguousarray(
        E.T.reshape(8, P, D).transpose(1, 0, 2).reshape(P, 8 * D)
    ).astype(NPBF16)
    shared = {"dyt": DyTp, "et": ETp, "dup": dup_pack}
    in_maps = []
    for b in range(B):
        m = dict(shared)
        embT = emb[b].T  # [d, t]
        for k in range(2):
            m[f"embdx{k}"] = np.ascontiguousarray(np.concatenate(
                [embT[k * P:(k + 1) * P, :], DxT[k * P:(k + 1) * P, :]],
                axis=1)).astype(NPBF16)
        m["emb"] = np.ascontiguousarray(
            emb[b].reshape(2, P, D).transpose(1, 0, 2).reshape(P, 2 * D)
        ).astype(NPBF16)
        in_maps.append(m)
    return in_maps


def kernel(embeddings, E, Dx, Dy, _use_f32r=True):
    in_maps = make_in_maps(embeddings, E, Dx, Dy)
    nc = _get_nc()
    res = run_bass_kernel_spmd(nc, in_maps, core_ids=list(range(B)))
    return np.stack([r["out"] for r in res.results], axis=0)
